# revision 15
# baseline (speedup 1.0000x reference)
"""Trainium2 Bass kernel for batched displacement-operator construction.

Math: for each alpha_b,
    Da[b] = diag(u) @ (V @ diag(exp(-i r lam)) @ V.T) @ diag(v)
with u_i = w^i, v_j = w^-j, w = i*alpha/|alpha|.

Parity reduction: the generator H (tridiagonal) anticommutes with the
parity operator Pi = diag((-1)^n), so M = V exp(-i r Lam) V^T is real on
even i-j and purely imaginary on odd i-j, and eigenpairs come in
(lam, v), (-lam, Pi v) pairs.  Writing Ue/Uo for the even/odd rows of
the positive-lambda eigenvectors (512x512 each):

    M[2i',2j']     = Cee = Ue diag(2 cos r lam+) Ue^T      (real)
    M[2i'+1,2j'+1] = Coo = Uo diag(2 cos r lam+) Uo^T      (real)
    M[2i',2j'+1]   = i*Seo,  Seo = Ue diag(-2 sin r lam+) Uo^T
    M[2i'+1,2j']   = i*Soe,  Soe = Seo^T

so the two full 1024^3 real matmuls of the direct method collapse to
three 512^3 matmuls + transposes (4x fewer MACs), and the symmetric
Cee/Coo factors skip their upper-right quarters (mirrored by PE
transposes) for another ~8% off the matmul stream.  The outer phase
w^(i-j) is Toeplitz; per parity block it is applied as an elementwise
multiply by a [128, 896] shifted-window table (host-precomputed), with
the even/odd column interleave done on-chip via stride-2 writes.
Outputs are bf16 (rel err ~2e-3, far under the 2e-2 gate), halving the
output DMA traffic.

Sharding: 16 alphas data-parallel over 8 cores (2 per core); Ue/Uo
replicated.  Both alphas share each LDWEIGHTS (4 moving streams per
stationary block).  Measured per-core body time ~23us (PE-roofline:
96 fp32r MMs ~20.5us + 32 transposes ~3.4us); DVE ~19us, ACT ~16us,
Pool ~4us, DMA-out 4.2MB ~15us all hide under PE.
"""

import sys

sys.path.insert(0, "/opt/trn_rl_repo")

import numpy as np

N = 1024
H = 512  # parity half-dimension
B = 16
NCORES = 8
APC = B // NCORES  # alphas per core
P = 128
HC = H // P  # chunks per half-dim (4)
NT = 512  # matmul free-dim / psum bank width (fp32)
TW = 896  # phase-window free size
OFF = 384  # phase-window offset: col t0 = OFF - 128*m

_cache = {}


def _build_module(reps=1, unroll=1):
    import contextlib

    import concourse.bacc as bacc
    import concourse.mybir as mybir
    import concourse.tile as tile

    f32 = mybir.dt.float32
    f32r = mybir.dt.float32r
    bf16 = mybir.dt.bfloat16

    nc = bacc.Bacc(
        "TRN2",
        target_bir_lowering=False,
        debug=False,
        num_devices=NCORES,
    )

    uet_d = nc.dram_tensor("uet", [H, H], f32, kind="ExternalInput")
    uot_d = nc.dram_tensor("uot", [H, H], f32, kind="ExternalInput")
    esc_d = nc.dram_tensor("esc", [P, APC * 2 * HC], f32, kind="ExternalInput")
    ph_d = nc.dram_tensor("ph", [P, APC * 6 * TW], bf16, kind="ExternalInput")
    outr_d = nc.dram_tensor("outr", [APC, N, N], bf16, kind="ExternalOutput")
    outi_d = nc.dram_tensor("outi", [APC, N, N], bf16, kind="ExternalOutput")

    with tile.TileContext(nc) as tc:
        with (
            tc.tile_pool(name="const", bufs=1) as cpool,
            tc.tile_pool(name="wts", bufs=2) as wpool,
            tc.tile_pool(name="work", bufs=2) as work,
            tc.tile_pool(name="outp", bufs=3) as outp,
            tc.tile_pool(name="ssbp", bufs=2) as ssbp,
            tc.tile_pool(name="mirp", bufs=1) as mirp,
            tc.tile_pool(name="psum", bufs=8, space="PSUM") as pp,
        ):
            esc = cpool.tile([P, APC * 2 * HC], f32)
            ph = cpool.tile([P, APC * 6 * TW], bf16)
            from concourse.masks import make_identity

            ident_f = cpool.tile([P, P], f32, name="ident_f")
            make_identity(nc, ident_f)

            uet = [
                cpool.tile([P, H], f32r, tag=f"uet{kc}", name=f"uet{kc}")
                for kc in range(HC)
            ]
            uot = [
                cpool.tile([P, H], f32r, tag=f"uot{kc}", name=f"uot{kc}")
                for kc in range(HC)
            ]

            nc.gpsimd.dma_start(esc[:], esc_d[:])
            nc.gpsimd.dma_start(ph[:], ph_d[:])
            # fp32r DRAM binding crashes the exec unit; DMA fp32 and cast.
            for kc in range(HC):
                tmp = work.tile([P, H], f32, tag="uin")
                nc.sync.dma_start(tmp[:], uet_d[kc * P : (kc + 1) * P, :])
                nc.vector.tensor_copy(uet[kc][:], tmp[:])
            for kc in range(HC):
                tmp = work.tile([P, H], f32, tag="uin")
                nc.sync.dma_start(tmp[:], uot_d[kc * P : (kc + 1) * P, :])
                nc.vector.tensor_copy(uot[kc][:], tmp[:])

            rep_ctx = (
                tc.For_i(0, reps, 1) if reps > 1 else contextlib.nullcontext()
            )
            with rep_ctx:
                for _u in range(unroll):
                    _emit_body(nc, tc, uet, uot, esc, ph, wpool, work, outp,
                               pp, outr_d, outi_d, mybir, ident_f, ssbp, mirp)

    nc.compile()
    return nc


def _emit_body(nc, tc, uet, uot, esc, ph, wpool, work, outp, pp,
               outr_d, outi_d, mybir, ident, ssbp, mirp):
    f32 = mybir.dt.float32
    f32r = mybir.dt.float32r
    bf16 = mybir.dt.bfloat16
    Alu = mybir.AluOpType
    Act = mybir.ActivationFunctionType

    # Moving operands for both alphas: diag-scaled rows of Ue^T / Uo^T.
    # lce = diag(2cos) Ue^T, lso = diag(-2sin) Uo^T, lco = diag(2cos) Uo^T
    lce = [[None] * HC for _ in range(APC)]
    lso = [[None] * HC for _ in range(APC)]
    lco = [[None] * HC for _ in range(APC)]
    for a in range(APC):
        for kc in range(HC):
            col_c = a * 2 * HC + kc
            col_s = a * 2 * HC + HC + kc
            tc_ = wpool.tile([P, H], f32r, tag=f"lce{kc}a{a}", name=f"lce{kc}a{a}")
            ts_ = wpool.tile([P, H], f32r, tag=f"lso{kc}a{a}", name=f"lso{kc}a{a}")
            to_ = wpool.tile([P, H], f32r, tag=f"lco{kc}a{a}", name=f"lco{kc}a{a}")
            nc.scalar.activation(tc_[:], uet[kc][:], Act.Copy,
                                 scale=esc[:, col_c : col_c + 1])
            nc.scalar.activation(ts_[:], uot[kc][:], Act.Copy,
                                 scale=esc[:, col_s : col_s + 1])
            nc.scalar.activation(to_[:], uot[kc][:], Act.Copy,
                                 scale=esc[:, col_c : col_c + 1])
            lce[a][kc] = tc_
            lso[a][kc] = ts_
            lco[a][kc] = to_

    def win(a, wi, m):
        t0 = a * 6 * TW + wi * TW + (OFF - P * m)
        return ph[:, t0 : t0 + NT]

    ssb = {}
    mir = {}
    # Even output rows 2*(m*128+p): even cols from Cee, odd from Seo.
    # Both alphas share each LDWEIGHTS: 4 moving streams per stationary.
    # Cee is symmetric: m=2,3 run first at full width; m=0,1 compute only
    # cols 0:256 and mirror the upper-right quarter by transposing the
    # lower-left tiles (saved to SBUF by ACT).
    for m in (2, 3, 0, 1):
        wmm = NT if m >= 2 else NT // 2
        pcee = [pp.tile([P, NT], f32, tag="ps", name=f"pcee{m}_{_a}")
                for _a in range(APC)]
        pseo = [pp.tile([P, NT], f32, tag="ps", name=f"pseo{m}_{_a}")
                for _a in range(APC)]
        for kc in range(HC):
            wap = uet[kc][:, m * P : (m + 1) * P]
            st = kc == 0
            sp = kc == HC - 1
            for a in range(APC):
                nc.tensor.matmul(pcee[a][:, 0:wmm], wap,
                                 lce[a][kc][:, 0:wmm], start=st, stop=sp)
                nc.tensor.matmul(pseo[a][:], wap, lso[a][kc][:],
                                 start=st, stop=sp)
        if m >= 2:
            for a in range(APC):
                mv = mirp.tile([P, NT // 2], f32, tag=f"mirc{m}a{a}",
                               name=f"mirc{m}a{a}")
                nc.scalar.activation(mv[:], pcee[a][:, 0 : NT // 2], Act.Copy)
                mir["c", m, a] = mv
        else:
            for a in range(APC):
                for i, msrc in enumerate((2, 3)):
                    nc.tensor.matmul(
                        pcee[a][:, NT // 2 + i * P : NT // 2 + (i + 1) * P],
                        mir["c", msrc, a][:, m * P : (m + 1) * P], ident[:],
                        is_transpose=True, start=True, stop=True,
                    )
        for a in range(APC):
            # SBUF copy of Seo chunk: transpose source for Soe AND the
            # Pool-readable operand for the odd-column phase mults (Pool
            # has no PSUM port).
            sb = ssbp.tile([P, NT], f32, tag=f"ssb{m}a{a}", name=f"ssb{m}a{a}")
            nc.scalar.activation(sb[:], pseo[a][:], Act.Copy)
            ssb[a, m] = sb

            ere = outp.tile([P, N], bf16, tag="ere")
            eim = outp.tile([P, N], bf16, tag="eim")
            nc.vector.tensor_tensor(ere[:, 0:N:2], pcee[a][:], win(a, 0, m),
                                    Alu.mult)
            nc.vector.tensor_tensor(eim[:, 0:N:2], pcee[a][:], win(a, 1, m),
                                    Alu.mult)
            nc.gpsimd.tensor_tensor(ere[:, 1:N:2], sb[:], win(a, 2, m),
                                    Alu.mult)
            nc.gpsimd.tensor_tensor(eim[:, 1:N:2], sb[:], win(a, 3, m),
                                    Alu.mult)
            nc.sync.dma_start(
                outr_d[a, 2 * m * P : 2 * (m + 1) * P : 2, :], ere[:]
            )
            nc.sync.dma_start(
                outi_d[a, 2 * m * P : 2 * (m + 1) * P : 2, :], eim[:]
            )

    # Odd output rows: odd cols from Coo, even cols from Soe = Seo^T.
    # Coo is symmetric: same triangle scheme as Cee.
    for m in (2, 3, 0, 1):
        wmm = NT if m >= 2 else NT // 2
        pcoo = [pp.tile([P, NT], f32, tag="ps", name=f"pcoo{m}_{_a}")
                for _a in range(APC)]
        psoe = [pp.tile([P, NT], f32, tag="ps", name=f"psoe{m}_{_a}")
                for _a in range(APC)]
        for kc in range(HC):
            wap = uot[kc][:, m * P : (m + 1) * P]
            st = kc == 0
            sp = kc == HC - 1
            for a in range(APC):
                nc.tensor.matmul(pcoo[a][:, 0:wmm], wap,
                                 lco[a][kc][:, 0:wmm], start=st, stop=sp)
        if m >= 2:
            for a in range(APC):
                mv = mirp.tile([P, NT // 2], f32, tag=f"miro{m}a{a}",
                               name=f"miro{m}a{a}")
                nc.scalar.activation(mv[:], pcoo[a][:, 0 : NT // 2], Act.Copy)
                mir["o", m, a] = mv
        else:
            for a in range(APC):
                for i, msrc in enumerate((2, 3)):
                    nc.tensor.matmul(
                        pcoo[a][:, NT // 2 + i * P : NT // 2 + (i + 1) * P],
                        mir["o", msrc, a][:, m * P : (m + 1) * P], ident[:],
                        is_transpose=True, start=True, stop=True,
                    )
        for a in range(APC):
            for q in range(HC):
                nc.tensor.matmul(
                    psoe[a][:, q * P : (q + 1) * P],
                    ssb[a, q][:, m * P : (m + 1) * P], ident[:],
                    is_transpose=True, start=True, stop=True,
                )
            ore = outp.tile([P, N], bf16, tag="ore")
            oim = outp.tile([P, N], bf16, tag="oim")
            nc.vector.tensor_tensor(ore[:, 1:N:2], pcoo[a][:], win(a, 0, m),
                                    Alu.mult)
            nc.vector.tensor_tensor(oim[:, 1:N:2], pcoo[a][:], win(a, 1, m),
                                    Alu.mult)
            nc.vector.tensor_tensor(ore[:, 0:N:2], psoe[a][:], win(a, 4, m),
                                    Alu.mult)
            nc.vector.tensor_tensor(oim[:, 0:N:2], psoe[a][:], win(a, 5, m),
                                    Alu.mult)
            nc.sync.dma_start(
                outr_d[a, 2 * m * P + 1 : 2 * (m + 1) * P : 2, :], ore[:]
            )
            nc.sync.dma_start(
                outi_d[a, 2 * m * P + 1 : 2 * (m + 1) * P : 2, :], oim[:]
            )


def _get_module():
    if "nc" not in _cache:
        _cache["nc"] = _build_module()
    return _cache["nc"]


def _host_precompute(alpha_real, alpha_imag, evals):
    """Per-alpha scalar/window tables, fp64 host math for the phases."""
    ar = np.asarray(alpha_real, np.float32)
    ai = np.asarray(alpha_imag, np.float32)
    ev = np.asarray(evals, np.float64)
    lamp = ev[H:]  # positive eigenvalues, ascending

    esc_all = np.empty((B, 2, HC, P), np.float32)  # (b, c2/s2, kc, p)
    ph_all = np.empty((B, 6, P, TW), np.float32)  # (b, window, p, t)

    prow = np.arange(P)[:, None]
    tcol = np.arange(TW)[None, :]
    idx = prow - tcol + OFF + (H - 1)  # into d-tables of length 2H-1
    d = np.arange(-(H - 1), H).astype(np.float64)

    for b in range(B):
        alpha = complex(float(ar[b]), float(ai[b]))
        r = np.float64(abs(np.complex64(alpha))) + np.float64(np.float32(1e-10))
        w = 1j * alpha / r

        c2 = (2.0 * np.cos(r * lamp)).astype(np.float32)
        s2 = (-2.0 * np.sin(r * lamp)).astype(np.float32)
        esc_all[b, 0] = c2.reshape(HC, P)
        esc_all[b, 1] = s2.reshape(HC, P)

        w2d = w ** (2 * d)
        w2dm = w ** (2 * d - 1)
        w2dp = w ** (2 * d + 1)
        tabs = (
            np.real(w2d), np.imag(w2d),
            -np.imag(w2dm), np.real(w2dm),
            -np.imag(w2dp), np.real(w2dp),
        )
        for wi, tab in enumerate(tabs):
            ph_all[b, wi] = tab.astype(np.float32)[idx]

    return esc_all, ph_all


def _make_in_maps(alpha_real, alpha_imag, evals, evecs):
    import ml_dtypes

    evecs_f = np.asarray(evecs, np.float32)
    uet_np = np.ascontiguousarray(evecs_f[0::2, H:].T)  # [k, i'] even sites
    uot_np = np.ascontiguousarray(evecs_f[1::2, H:].T)
    esc_all, ph_all = _host_precompute(alpha_real, alpha_imag, evals)

    in_maps = []
    for c in range(NCORES):
        bs = [c * APC + a for a in range(APC)]
        esc = np.empty((P, APC * 2 * HC), np.float32)
        ph = np.empty((P, APC * 6 * TW), ml_dtypes.bfloat16)
        for a, b in enumerate(bs):
            for which in range(2):
                cols = a * 2 * HC + which * HC
                esc[:, cols : cols + HC] = esc_all[b, which].T
            for wi in range(6):
                wbase = (a * 6 + wi) * TW
                ph[:, wbase : wbase + TW] = ph_all[b, wi]
        in_maps.append({"uet": uet_np, "uot": uot_np, "esc": esc, "ph": ph})
    return in_maps


def kernel(alpha_real, alpha_imag, evals, evecs):
    from concourse import bass_utils

    nc = _get_module()

    in_maps = _make_in_maps(alpha_real, alpha_imag, evals, evecs)

    res = bass_utils.run_bass_kernel_spmd(
        nc, in_maps, core_ids=list(range(NCORES))
    )

    out = np.empty((B, N, N), np.complex64)
    for c in range(NCORES):
        outr = np.asarray(res.results[c]["outr"], dtype=np.float32)
        outi = np.asarray(res.results[c]["outi"], dtype=np.float32)
        for a in range(APC):
            b = c * APC + a
            out.real[b] = outr[a]
            out.imag[b] = outi[a]
    return out
